# revision 20
# baseline (speedup 1.0000x reference)
"""DiceLoss (softmax + one-hot gather + per-sample dice) on 8 trn2 cores.

Sharding: pure data-parallel over the batch dim (N=32 -> 4 samples/core).
Each core streams its 4 samples, computing per-pixel
    p = exp(x_t) / sum_c exp(x_c)
and accumulating per-partition sums of p. The host finishes with the
(tiny) dice formula. The softmax prob sum over classes is identically 1
per pixel, so cardinality = 2*H*W analytically.

Per-core layout: partitions = (4 samples x 32 pixel-blocks) = 128; free
dim = 8192 pixels per block, processed in 4 chunks of 2048.

Engine assignment (GPSIMD stays idle: its SBUF port lock vs DVE was the
previous bottleneck -- concurrent DVE ops ran 25x slow):
  - DMA:  x as one 4MiB HWDGE transfer per chunk (sync ring);
          t (int32) + out on the scalar HWDGE ring
  - ACT:  exp(x_c) per class (bf16), ln(D)/ln(N) straight from PSUM,
          final exp(lnN-lnD) with accum_out per-partition sums
  - DVE:  fused one-hot numerator U_c=(t==c)*e_c via scalar_tensor_tensor,
          z = lnN - lnD (bf16 2x mode)
  - PE:   class-sum trees D=sum_c E_c, N=sum_c U_c as identity-weight
          matmuls accumulating in PSUM (4 matmuls per 512-col bank)
"""

import os
import sys

import numpy as np


def _ensure_concourse():
    try:
        import concourse.bass  # noqa: F401
    except ImportError:
        for p in (
            "/opt/trn_rl_repo",
            os.path.expanduser("~/.axon_site/_ro/trn_rl_repo"),
        ):
            if os.path.isdir(p) and p not in sys.path:
                sys.path.insert(0, p)


_ensure_concourse()

import concourse.bacc as bacc  # noqa: E402
import concourse.bass as bass  # noqa: E402
import concourse.mybir as mybir  # noqa: E402
from concourse.bass_utils import run_bass_kernel_spmd  # noqa: E402
from concourse.tile import TileContext  # noqa: E402

N, C, H, W = 32, 4, 512, 512
NCORES = 8
SPC = N // NCORES  # samples per core = 4
PB = 32  # pixel blocks per sample (partition sub-dim)
P = SPC * PB  # 128 partitions
FTOT = H * W // PB  # 8192 free-dim pixels per block
FC = 2048  # chunk size along free dim
NCHUNK = FTOT // FC  # 4
NACC = NCHUNK + 1  # last chunk finishes in 2 halves -> one extra acc column
HB = 1024  # PSUM half-chunk (2 banks per [128, HB] fp32 tile)
MM = 512  # matmul output columns per instruction (1 PSUM bank)
EPS = 1e-6

_cache = {}
LAST_EXEC_NS = None
LAST_RESULT = None


def _build():
    nc = bacc.Bacc(None)
    # x arrives class-outermost so the (s, pb) partition dims are adjacent in
    # HBM -> one mergeable partition stride -> one 4MiB DMA per chunk.
    x = nc.dram_tensor("x", [C, SPC, H, W], mybir.dt.float32, kind="ExternalInput")
    t = nc.dram_tensor("t", [SPC, 1, H, W], mybir.dt.int32, kind="ExternalInput")
    eye_d = nc.dram_tensor("eye", [P, P], mybir.dt.bfloat16, kind="ExternalInput")
    out = nc.dram_tensor("out", [P, NACC], mybir.dt.float32, kind="ExternalOutput")

    # partition = (s, pb); free = (c, fh*W + w) for x, (fh*W + w) for t
    xv = x[:].rearrange("c s (pb fh) w -> (s pb) c (fh w)", pb=PB)  # [128, 4, 8192]
    tv = t[:].rearrange("s o (pb fh) w -> (s o pb) (fh w)", pb=PB)  # [128, 8192]

    AF = mybir.ActivationFunctionType
    OP = mybir.AluOpType
    f32 = mybir.dt.float32
    bf16 = mybir.dt.bfloat16

    with TileContext(nc) as tc:
        with (
            tc.tile_pool(name="const", bufs=1) as constp,
            tc.tile_pool(name="accp", bufs=1) as accp,
            tc.tile_pool(name="xp", bufs=3) as xp,
            tc.tile_pool(name="tp", bufs=2) as tp,
            tc.tile_pool(name="ep", bufs=2) as ep,
            tc.tile_pool(name="up", bufs=2) as up,
            tc.tile_pool(name="lp", bufs=2) as lp,
            tc.tile_pool(name="psum", bufs=2, space="PSUM") as pp,
        ):
            eye = constp.tile([P, P], bf16, name="eye")
            acc = accp.tile([P, NACC], f32, name="acc")

            state = []  # per-chunk tiles to finish one chunk later

            def start_chunk(k):
                sl = slice(k * FC, (k + 1) * FC)
                X4 = xp.tile([P, C, FC], f32, tag="x", name=f"X4_{k}")
                T = tp.tile([P, FC], mybir.dt.int32, tag="t", name=f"T_{k}")
                E = [
                    ep.tile([P, FC], bf16, tag=f"e{c}", name=f"E{c}_{k}")
                    for c in range(C)
                ]
                U = [
                    up.tile([P, FC], bf16, tag=f"u{c}", name=f"U{c}_{k}")
                    for c in range(C)
                ]
                if k == 0:
                    # per-class loads so compute starts after ~1MiB, not 4
                    for c in range(C):
                        nc.sync.dma_start(X4[:, c, :], xv[:, c, sl])
                else:
                    nc.sync.dma_start(X4[:], xv[:, :, sl])
                nc.scalar.dma_start(T[:], tv[:, sl])
                if k == 0:
                    nc.scalar.dma_start(eye[:], eye_d[:])

                # one-hot gather of the LOGIT on DVE: U_c = (t == c) * x_c.
                # Depends only on the DMA, so it runs parallel to ACT's exps.
                for c in range(C):
                    nc.vector.scalar_tensor_tensor(
                        U[c][:], T[:], float(c), X4[:, c, :], OP.is_equal, OP.mult
                    )
                for c in range(C):
                    nc.scalar.activation(E[c][:], X4[:, c, :], AF.Exp)
                # x_t = sum_c U_c (DVE tree, in-place)
                nc.vector.tensor_tensor(U[0][:], U[0][:], U[1][:], OP.add)
                nc.vector.tensor_tensor(U[2][:], U[2][:], U[3][:], OP.add)
                nc.vector.tensor_tensor(U[0][:], U[0][:], U[2][:], OP.add)
                # denominator sum on the tensor engine: D = sum_c E_c
                D = pp.tile([P, FC], f32, tag="d", name=f"D_{k}")
                for j in range(FC // MM):
                    for c in range(C):
                        nc.tensor.matmul(
                            D[:, j * MM : (j + 1) * MM],
                            eye[:],
                            E[c][:, j * MM : (j + 1) * MM],
                            start=(c == 0),
                            stop=(c == C - 1),
                        )
                state.append((k, D, U[0]))

            def finish_chunk():
                k, D, XT = state.pop(0)
                LND = lp.tile([P, FC], bf16, tag="lnd", name=f"LND_{k}")
                ZE = lp.tile([P, FC], bf16, tag="ze", name=f"ZE_{k}")
                # last chunk: finish in halves for a shorter serial tail
                halves = 2 if k == NCHUNK - 1 else 1
                hw_ = FC // halves
                for h in range(halves):
                    hs = slice(h * hw_, (h + 1) * hw_)
                    nc.scalar.activation(LND[:, hs], D[:, hs], AF.Ln)
                    # z = x_t - ln D; p = exp(z), accumulated per partition
                    nc.vector.tensor_tensor(XT[:, hs], XT[:, hs], LND[:, hs], OP.subtract)
                    nc.scalar.activation(
                        ZE[:, hs], XT[:, hs], AF.Exp, accum_out=acc[:, k + h : k + h + 1]
                    )
                nc.sync.dma_start(out[:, k : k + 1], acc[:, k : k + 1])
                if halves == 2:
                    nc.sync.dma_start(out[:, k + 1 : k + 2], acc[:, k + 1 : k + 2])

            for k in range(NCHUNK):
                start_chunk(k)
                if k >= 1:
                    finish_chunk()
            finish_chunk()
    nc.compile()  # bacc passes: split sync waits, fill ISA bytes, ...
    _force_single_act_table(nc)
    return nc


def _force_single_act_table(nc):
    """The bacc pass picks the first act-table set per function (Exp->0,
    Ln->5), reloading tables on every switch (~2.7us each). Both live in
    set 6 (natural_log_exp_and_others): retarget and dedupe the loads."""
    both = 6
    for blk in nc.main_func.blocks:
        keep = []
        last = None
        for ins in blk.instructions:
            if type(ins).__name__ == "InstLoadActFuncSet":
                if ins.act_func_set_id in (0, 5):
                    ins.act_func_set_id = both
                if ins.sync_info is None and last == ins.act_func_set_id:
                    continue  # redundant reload
                last = ins.act_func_set_id
            keep.append(ins)
        blk.instructions[:] = keep


def kernel(input, target):
    global LAST_EXEC_NS
    nc = _cache.get("nc")
    if nc is None:
        nc = _cache.setdefault("nc", _build())

    input = np.asarray(input)
    target = np.asarray(target)
    bf16_np = mybir.dt.np(mybir.dt.bfloat16)
    eye_np = np.eye(P, dtype=np.float32).astype(bf16_np)
    in_maps = []
    for i in range(NCORES):
        in_maps.append(
            {
                "x": np.ascontiguousarray(
                    input[i * SPC : (i + 1) * SPC].transpose(1, 0, 2, 3),
                    dtype=np.float32,
                ),
                "t": np.ascontiguousarray(
                    target[i * SPC : (i + 1) * SPC], dtype=np.int32
                ),
                "eye": eye_np,
            }
        )
    res = run_bass_kernel_spmd(nc, in_maps, list(range(NCORES)))
    LAST_EXEC_NS = res.exec_time_ns
    globals()["LAST_RESULT"] = res

    Is = []
    for i in range(NCORES):
        o = np.asarray(res.results[i]["out"], dtype=np.float64)  # [128, NCHUNK]
        Is.append(o.sum(axis=1).reshape(SPC, PB).sum(axis=1))
    intersection = np.concatenate(Is)  # [32]
    hw = float(H * W)
    dice = 2.0 * intersection / (hw + hw + EPS)
    return np.float32(np.mean(1.0 - dice))


# revision 22
# speedup vs baseline: 1.0262x; 1.0262x over previous
"""DiceLoss (softmax + one-hot gather + per-sample dice) on 8 trn2 cores.

Sharding: pure data-parallel over the batch dim (N=32 -> 4 samples/core).
Each core streams its 4 samples, computing per-pixel
    p = exp(x_t) / sum_c exp(x_c)
and accumulating per-partition sums of p. The host finishes with the
(tiny) dice formula. The softmax prob sum over classes is identically 1
per pixel, so cardinality = 2*H*W analytically.

Per-core layout: partitions = (4 samples x 32 pixel-blocks) = 128; free
dim = 8192 pixels per block, processed in 4 chunks of 2048.

Engine assignment (GPSIMD stays idle: its SBUF port lock vs DVE was the
previous bottleneck -- concurrent DVE ops ran 25x slow):
  - DMA:  x as one 4MiB HWDGE transfer per chunk (sync ring);
          t (int32) + out on the scalar HWDGE ring
  - ACT:  exp(x_c) per class (bf16), ln(D)/ln(N) straight from PSUM,
          final exp(lnN-lnD) with accum_out per-partition sums
  - DVE:  fused one-hot numerator U_c=(t==c)*e_c via scalar_tensor_tensor,
          z = lnN - lnD (bf16 2x mode)
  - PE:   class-sum trees D=sum_c E_c, N=sum_c U_c as identity-weight
          matmuls accumulating in PSUM (4 matmuls per 512-col bank)
"""

import os
import sys

import numpy as np


def _ensure_concourse():
    try:
        import concourse.bass  # noqa: F401
    except ImportError:
        for p in (
            "/opt/trn_rl_repo",
            os.path.expanduser("~/.axon_site/_ro/trn_rl_repo"),
        ):
            if os.path.isdir(p) and p not in sys.path:
                sys.path.insert(0, p)


_ensure_concourse()

import concourse.bacc as bacc  # noqa: E402
import concourse.bass as bass  # noqa: E402
import concourse.mybir as mybir  # noqa: E402
from concourse.bass_utils import run_bass_kernel_spmd  # noqa: E402
from concourse.tile import TileContext  # noqa: E402

N, C, H, W = 32, 4, 512, 512
NCORES = 8
SPC = N // NCORES  # samples per core = 4
PB = 32  # pixel blocks per sample (partition sub-dim)
P = SPC * PB  # 128 partitions
FTOT = H * W // PB  # 8192 free-dim pixels per block
FC = 2048  # chunk size along free dim
NCHUNK = FTOT // FC  # 4
NACC = NCHUNK + 1  # last chunk finishes in 2 halves -> one extra acc column
HB = 1024  # PSUM half-chunk (2 banks per [128, HB] fp32 tile)
MM = 512  # matmul output columns per instruction (1 PSUM bank)
EPS = 1e-6

_cache = {}
LAST_EXEC_NS = None
LAST_RESULT = None


def _build():
    nc = bacc.Bacc(None)
    # x arrives class-outermost so the (s, pb) partition dims are adjacent in
    # HBM -> one mergeable partition stride -> one 4MiB DMA per chunk.
    x = nc.dram_tensor("x", [C, SPC, H, W], mybir.dt.float32, kind="ExternalInput")
    t = nc.dram_tensor("t", [SPC, 1, H, W], mybir.dt.int32, kind="ExternalInput")
    eye_d = nc.dram_tensor("eye", [P, P], mybir.dt.bfloat16, kind="ExternalInput")
    out = nc.dram_tensor("out", [P, NACC], mybir.dt.float32, kind="ExternalOutput")

    # partition = (s, pb); free = (c, fh*W + w) for x, (fh*W + w) for t
    xv = x[:].rearrange("c s (pb fh) w -> (s pb) c (fh w)", pb=PB)  # [128, 4, 8192]
    tv = t[:].rearrange("s o (pb fh) w -> (s o pb) (fh w)", pb=PB)  # [128, 8192]

    AF = mybir.ActivationFunctionType
    OP = mybir.AluOpType
    f32 = mybir.dt.float32
    bf16 = mybir.dt.bfloat16

    with TileContext(nc) as tc:
        with (
            tc.tile_pool(name="const", bufs=1) as constp,
            tc.tile_pool(name="accp", bufs=1) as accp,
            tc.tile_pool(name="xp", bufs=2) as xp,
            tc.tile_pool(name="tp", bufs=2) as tp,
            tc.tile_pool(name="ep", bufs=2) as ep,
            tc.tile_pool(name="up", bufs=2) as up,
            tc.tile_pool(name="lp", bufs=2) as lp,
            tc.tile_pool(name="psum", bufs=2, space="PSUM") as pp,
        ):
            eye = constp.tile([P, P], bf16, name="eye")
            acc = accp.tile([P, NACC], f32, name="acc")

            state = []  # per-chunk tiles to finish one chunk later

            def start_chunk(k):
                sl = slice(k * FC, (k + 1) * FC)
                X4 = xp.tile([P, C, FC], f32, tag="x", name=f"X4_{k}")
                T = tp.tile([P, FC], mybir.dt.int32, tag="t", name=f"T_{k}")
                E = [
                    ep.tile([P, FC], bf16, tag=f"e{c}", name=f"E{c}_{k}")
                    for c in range(C)
                ]
                U = [
                    up.tile([P, FC], bf16, tag=f"u{c}", name=f"U{c}_{k}")
                    for c in range(C)
                ]
                if k == 0:
                    # per-class loads so compute starts after ~1MiB, not 4
                    for c in range(C):
                        nc.sync.dma_start(X4[:, c, :], xv[:, c, sl])
                else:
                    nc.sync.dma_start(X4[:], xv[:, :, sl])
                nc.scalar.dma_start(T[:], tv[:, sl])
                if k == 0:
                    nc.scalar.dma_start(eye[:], eye_d[:])

                # one-hot gather of the LOGIT on DVE: U_c = (t == c) * x_c.
                # Depends only on the DMA, so it runs parallel to ACT's exps.
                for c in range(C):
                    nc.vector.scalar_tensor_tensor(
                        U[c][:], T[:], float(c), X4[:, c, :], OP.is_equal, OP.mult
                    )
                for c in range(C):
                    nc.scalar.activation(E[c][:], X4[:, c, :], AF.Exp)
                # x_t = sum_c U_c (DVE tree, in-place)
                nc.vector.tensor_tensor(U[0][:], U[0][:], U[1][:], OP.add)
                nc.vector.tensor_tensor(U[2][:], U[2][:], U[3][:], OP.add)
                nc.vector.tensor_tensor(U[0][:], U[0][:], U[2][:], OP.add)
                # denominator sum on the tensor engine: D = sum_c E_c
                D = pp.tile([P, FC], f32, tag="d", name=f"D_{k}")
                for j in range(FC // MM):
                    for c in range(C):
                        nc.tensor.matmul(
                            D[:, j * MM : (j + 1) * MM],
                            eye[:],
                            E[c][:, j * MM : (j + 1) * MM],
                            start=(c == 0),
                            stop=(c == C - 1),
                        )
                state.append((k, D, U[0]))

            def finish_chunk():
                k, D, XT = state.pop(0)
                LND = lp.tile([P, FC], bf16, tag="lnd", name=f"LND_{k}")
                ZE = lp.tile([P, FC], bf16, tag="ze", name=f"ZE_{k}")
                nc.scalar.activation(LND[:], D[:], AF.Ln)
                # z = x_t - ln D; p = exp(z), accumulated per partition
                nc.vector.tensor_tensor(XT[:], XT[:], LND[:], OP.subtract)
                nc.scalar.activation(
                    ZE[:], XT[:], AF.Exp, accum_out=acc[:, k : k + 1]
                )

            for k in range(NCHUNK):
                start_chunk(k)
                if k >= 1:
                    finish_chunk()
            finish_chunk()
            nc.scalar.dma_start(out[:, :NCHUNK], acc[:, :NCHUNK])
    nc.compile()  # bacc passes: split sync waits, fill ISA bytes, ...
    _force_single_act_table(nc)
    return nc


def _force_single_act_table(nc):
    """The bacc pass picks the first act-table set per function (Exp->0,
    Ln->5), reloading tables on every switch (~2.7us each). Both live in
    set 6 (natural_log_exp_and_others): retarget and dedupe the loads."""
    both = 6
    for blk in nc.main_func.blocks:
        keep = []
        last = None
        for ins in blk.instructions:
            if type(ins).__name__ == "InstLoadActFuncSet":
                if ins.act_func_set_id in (0, 5):
                    ins.act_func_set_id = both
                if ins.sync_info is None and last == ins.act_func_set_id:
                    continue  # redundant reload
                last = ins.act_func_set_id
            keep.append(ins)
        blk.instructions[:] = keep


def kernel(input, target):
    global LAST_EXEC_NS
    nc = _cache.get("nc")
    if nc is None:
        nc = _cache.setdefault("nc", _build())

    input = np.asarray(input)
    target = np.asarray(target)
    bf16_np = mybir.dt.np(mybir.dt.bfloat16)
    eye_np = np.eye(P, dtype=np.float32).astype(bf16_np)
    in_maps = []
    for i in range(NCORES):
        in_maps.append(
            {
                "x": np.ascontiguousarray(
                    input[i * SPC : (i + 1) * SPC].transpose(1, 0, 2, 3),
                    dtype=np.float32,
                ),
                "t": np.ascontiguousarray(
                    target[i * SPC : (i + 1) * SPC], dtype=np.int32
                ),
                "eye": eye_np,
            }
        )
    res = run_bass_kernel_spmd(nc, in_maps, list(range(NCORES)))
    LAST_EXEC_NS = res.exec_time_ns
    globals()["LAST_RESULT"] = res

    Is = []
    for i in range(NCORES):
        o = np.asarray(res.results[i]["out"], dtype=np.float64)  # [128, NCHUNK]
        Is.append(o.sum(axis=1).reshape(SPC, PB).sum(axis=1))
    intersection = np.concatenate(Is)  # [32]
    hw = float(H * W)
    dice = 2.0 * intersection / (hw + hw + EPS)
    return np.float32(np.mean(1.0 - dice))


# revision 23
# speedup vs baseline: 1.1069x; 1.0787x over previous
"""DiceLoss (softmax + one-hot gather + per-sample dice) on 8 trn2 cores.

Sharding: pure data-parallel over the batch dim (N=32 -> 4 samples/core).
Each core streams its 4 samples, computing per-pixel
    p = exp(x_t) / sum_c exp(x_c)
and accumulating per-partition sums of p. The host finishes with the
(tiny) dice formula. The softmax prob sum over classes is identically 1
per pixel, so cardinality = 2*H*W analytically.

Per-core layout: partitions = (4 samples x 32 pixel-blocks) = 128; free
dim = 8192 pixels per block, processed in 4 chunks of 2048.

Engine assignment (GPSIMD stays idle: its SBUF port lock vs DVE was the
previous bottleneck -- concurrent DVE ops ran 25x slow):
  - DMA:  x as one 4MiB HWDGE transfer per chunk (sync ring);
          t (int32) + out on the scalar HWDGE ring
  - ACT:  exp(x_c) per class (bf16), ln(D)/ln(N) straight from PSUM,
          final exp(lnN-lnD) with accum_out per-partition sums
  - DVE:  fused one-hot numerator U_c=(t==c)*e_c via scalar_tensor_tensor,
          z = lnN - lnD (bf16 2x mode)
  - PE:   class-sum trees D=sum_c E_c, N=sum_c U_c as identity-weight
          matmuls accumulating in PSUM (4 matmuls per 512-col bank)
"""

import os
import sys

import numpy as np


def _ensure_concourse():
    try:
        import concourse.bass  # noqa: F401
    except ImportError:
        for p in (
            "/opt/trn_rl_repo",
            os.path.expanduser("~/.axon_site/_ro/trn_rl_repo"),
        ):
            if os.path.isdir(p) and p not in sys.path:
                sys.path.insert(0, p)


_ensure_concourse()

import concourse.bacc as bacc  # noqa: E402
import concourse.bass as bass  # noqa: E402
import concourse.mybir as mybir  # noqa: E402
from concourse.bass_utils import run_bass_kernel_spmd  # noqa: E402
from concourse.tile import TileContext  # noqa: E402

N, C, H, W = 32, 4, 512, 512
NCORES = 8
SPC = N // NCORES  # samples per core = 4
PB = 32  # pixel blocks per sample (partition sub-dim)
P = SPC * PB  # 128 partitions
FTOT = H * W // PB  # 8192 free-dim pixels per block
FC = 2048  # chunk size along free dim
NCHUNK = FTOT // FC  # 4
NACC = NCHUNK + 1  # last chunk finishes in 2 halves -> one extra acc column
HB = 1024  # PSUM half-chunk (2 banks per [128, HB] fp32 tile)
MM = 512  # matmul output columns per instruction (1 PSUM bank)
EPS = 1e-6

_cache = {}
LAST_EXEC_NS = None
LAST_RESULT = None


def _build():
    nc = bacc.Bacc(None)
    # x arrives class-outermost so the (s, pb) partition dims are adjacent in
    # HBM -> one mergeable partition stride -> one 4MiB DMA per chunk.
    x = nc.dram_tensor("x", [C, SPC, H, W], mybir.dt.float32, kind="ExternalInput")
    t = nc.dram_tensor("t", [SPC, 1, H, W], mybir.dt.int32, kind="ExternalInput")
    eye_d = nc.dram_tensor("eye", [P, P], mybir.dt.bfloat16, kind="ExternalInput")
    out = nc.dram_tensor("out", [P, NACC], mybir.dt.float32, kind="ExternalOutput")

    # partition = (s, pb); free = (c, fh*W + w) for x, (fh*W + w) for t
    xv = x[:].rearrange("c s (pb fh) w -> (s pb) c (fh w)", pb=PB)  # [128, 4, 8192]
    tv = t[:].rearrange("s o (pb fh) w -> (s o pb) (fh w)", pb=PB)  # [128, 8192]

    AF = mybir.ActivationFunctionType
    OP = mybir.AluOpType
    f32 = mybir.dt.float32
    bf16 = mybir.dt.bfloat16

    with TileContext(nc) as tc:
        with (
            tc.tile_pool(name="const", bufs=1) as constp,
            tc.tile_pool(name="accp", bufs=1) as accp,
            tc.tile_pool(name="xp", bufs=2) as xp,
            tc.tile_pool(name="tp", bufs=2) as tp,
            tc.tile_pool(name="ep", bufs=2) as ep,
            tc.tile_pool(name="up", bufs=2) as up,
            tc.tile_pool(name="lp", bufs=2) as lp,
            tc.tile_pool(name="psum", bufs=2, space="PSUM") as pp,
        ):
            eye = constp.tile([P, P], bf16, name="eye")
            acc = accp.tile([P, NACC], f32, name="acc")

            state = []  # per-chunk tiles to finish one chunk later

            def start_chunk(k):
                sl = slice(k * FC, (k + 1) * FC)
                X4 = xp.tile([P, C, FC], f32, tag="x", name=f"X4_{k}")
                T = tp.tile([P, FC], mybir.dt.int32, tag="t", name=f"T_{k}")
                E = [
                    ep.tile([P, FC], bf16, tag=f"e{c}", name=f"E{c}_{k}")
                    for c in range(C)
                ]
                U = [
                    up.tile([P, FC], bf16, tag=f"u{c}", name=f"U{c}_{k}")
                    for c in range(C)
                ]
                if k == 0:
                    # per-class loads so compute starts after ~1MiB, not 4
                    for c in range(C):
                        nc.sync.dma_start(X4[:, c, :], xv[:, c, sl])
                else:
                    # class pairs: downstream ops on c0/c1 start a transfer early
                    nc.sync.dma_start(X4[:, 0:2, :], xv[:, 0:2, sl])
                    nc.sync.dma_start(X4[:, 2:4, :], xv[:, 2:4, sl])
                nc.scalar.dma_start(T[:], tv[:, sl])
                if k == 0:
                    nc.scalar.dma_start(eye[:], eye_d[:])

                # one-hot gather of the LOGIT on DVE: U_c = (t == c) * x_c.
                # Depends only on the DMA, so it runs parallel to ACT's exps.
                for c in range(C):
                    nc.vector.scalar_tensor_tensor(
                        U[c][:], T[:], float(c), X4[:, c, :], OP.is_equal, OP.mult
                    )
                for c in range(C):
                    nc.scalar.activation(E[c][:], X4[:, c, :], AF.Exp)
                # x_t = sum_c U_c (DVE tree, in-place)
                nc.vector.tensor_tensor(U[0][:], U[0][:], U[1][:], OP.add)
                nc.vector.tensor_tensor(U[2][:], U[2][:], U[3][:], OP.add)
                nc.vector.tensor_tensor(U[0][:], U[0][:], U[2][:], OP.add)
                # denominator sum on the tensor engine: D = sum_c E_c
                D = pp.tile([P, FC], f32, tag="d", name=f"D_{k}")
                for j in range(FC // MM):
                    for c in range(C):
                        nc.tensor.matmul(
                            D[:, j * MM : (j + 1) * MM],
                            eye[:],
                            E[c][:, j * MM : (j + 1) * MM],
                            start=(c == 0),
                            stop=(c == C - 1),
                        )
                state.append((k, D, U[0]))

            def finish_chunk():
                k, D, XT = state.pop(0)
                LND = lp.tile([P, FC], bf16, tag="lnd", name=f"LND_{k}")
                ZE = lp.tile([P, FC], bf16, tag="ze", name=f"ZE_{k}")
                nc.scalar.activation(LND[:], D[:], AF.Ln)
                # z = x_t - ln D; p = exp(z), accumulated per partition
                nc.vector.tensor_tensor(XT[:], XT[:], LND[:], OP.subtract)
                nc.scalar.activation(
                    ZE[:], XT[:], AF.Exp, accum_out=acc[:, k : k + 1]
                )

            for k in range(NCHUNK):
                start_chunk(k)
                if k >= 1:
                    finish_chunk()
            finish_chunk()
            nc.scalar.dma_start(out[:, :NCHUNK], acc[:, :NCHUNK])
    nc.compile()  # bacc passes: split sync waits, fill ISA bytes, ...
    _force_single_act_table(nc)
    return nc


def _force_single_act_table(nc):
    """The bacc pass picks the first act-table set per function (Exp->0,
    Ln->5), reloading tables on every switch (~2.7us each). Both live in
    set 6 (natural_log_exp_and_others): retarget and dedupe the loads."""
    both = 6
    for blk in nc.main_func.blocks:
        keep = []
        last = None
        for ins in blk.instructions:
            if type(ins).__name__ == "InstLoadActFuncSet":
                if ins.act_func_set_id in (0, 5):
                    ins.act_func_set_id = both
                if ins.sync_info is None and last == ins.act_func_set_id:
                    continue  # redundant reload
                last = ins.act_func_set_id
            keep.append(ins)
        blk.instructions[:] = keep


def kernel(input, target):
    global LAST_EXEC_NS
    nc = _cache.get("nc")
    if nc is None:
        nc = _cache.setdefault("nc", _build())

    input = np.asarray(input)
    target = np.asarray(target)
    bf16_np = mybir.dt.np(mybir.dt.bfloat16)
    eye_np = np.eye(P, dtype=np.float32).astype(bf16_np)
    in_maps = []
    for i in range(NCORES):
        in_maps.append(
            {
                "x": np.ascontiguousarray(
                    input[i * SPC : (i + 1) * SPC].transpose(1, 0, 2, 3),
                    dtype=np.float32,
                ),
                "t": np.ascontiguousarray(
                    target[i * SPC : (i + 1) * SPC], dtype=np.int32
                ),
                "eye": eye_np,
            }
        )
    res = run_bass_kernel_spmd(nc, in_maps, list(range(NCORES)))
    LAST_EXEC_NS = res.exec_time_ns
    globals()["LAST_RESULT"] = res

    Is = []
    for i in range(NCORES):
        o = np.asarray(res.results[i]["out"], dtype=np.float64)  # [128, NCHUNK]
        Is.append(o.sum(axis=1).reshape(SPC, PB).sum(axis=1))
    intersection = np.concatenate(Is)  # [32]
    hw = float(H * W)
    dice = 2.0 * intersection / (hw + hw + EPS)
    return np.float32(np.mean(1.0 - dice))


# revision 24
# speedup vs baseline: 1.1160x; 1.0082x over previous
"""DiceLoss (softmax + one-hot gather + per-sample dice) on 8 trn2 cores.

Sharding: pure data-parallel over the batch dim (N=32 -> 4 samples/core).
Each core streams its 4 samples, computing per-pixel
    p = exp(x_t) / sum_c exp(x_c)
and accumulating per-partition sums of p. The host finishes with the
(tiny) dice formula. The softmax prob sum over classes is identically 1
per pixel, so cardinality = 2*H*W analytically.

Per-core layout: partitions = (4 samples x 32 pixel-blocks) = 128; free
dim = 8192 pixels per block, processed in 4 chunks of 2048.

Engine assignment (GPSIMD stays idle: its SBUF port lock vs DVE was the
previous bottleneck -- concurrent DVE ops ran 25x slow):
  - DMA:  x as one 4MiB HWDGE transfer per chunk (sync ring);
          t (int32) + out on the scalar HWDGE ring
  - ACT:  exp(x_c) per class (bf16), ln(D)/ln(N) straight from PSUM,
          final exp(lnN-lnD) with accum_out per-partition sums
  - DVE:  fused one-hot numerator U_c=(t==c)*e_c via scalar_tensor_tensor,
          z = lnN - lnD (bf16 2x mode)
  - PE:   class-sum trees D=sum_c E_c, N=sum_c U_c as identity-weight
          matmuls accumulating in PSUM (4 matmuls per 512-col bank)
"""

import os
import sys

import numpy as np


def _ensure_concourse():
    try:
        import concourse.bass  # noqa: F401
    except ImportError:
        for p in (
            "/opt/trn_rl_repo",
            os.path.expanduser("~/.axon_site/_ro/trn_rl_repo"),
        ):
            if os.path.isdir(p) and p not in sys.path:
                sys.path.insert(0, p)


_ensure_concourse()

import concourse.bacc as bacc  # noqa: E402
import concourse.bass as bass  # noqa: E402
import concourse.mybir as mybir  # noqa: E402
from concourse.bass_utils import run_bass_kernel_spmd  # noqa: E402
from concourse.tile import TileContext  # noqa: E402

N, C, H, W = 32, 4, 512, 512
NCORES = 8
SPC = N // NCORES  # samples per core = 4
PB = 32  # pixel blocks per sample (partition sub-dim)
P = SPC * PB  # 128 partitions
FTOT = H * W // PB  # 8192 free-dim pixels per block
FC = 2048  # chunk size along free dim
NCHUNK = FTOT // FC  # 4
NACC = NCHUNK + 1  # last chunk finishes in 2 halves -> one extra acc column
HB = 1024  # PSUM half-chunk (2 banks per [128, HB] fp32 tile)
MM = 512  # matmul output columns per instruction (1 PSUM bank)
EPS = 1e-6

_cache = {}
LAST_EXEC_NS = None
LAST_RESULT = None


def _build():
    nc = bacc.Bacc(None)
    # x arrives class-outermost so the (s, pb) partition dims are adjacent in
    # HBM -> one mergeable partition stride -> one 4MiB DMA per chunk.
    x = nc.dram_tensor("x", [C, SPC, H, W], mybir.dt.float32, kind="ExternalInput")
    t = nc.dram_tensor("t", [SPC, 1, H, W], mybir.dt.int32, kind="ExternalInput")
    eye_d = nc.dram_tensor("eye", [P, P], mybir.dt.bfloat16, kind="ExternalInput")
    out = nc.dram_tensor("out", [P, NACC], mybir.dt.float32, kind="ExternalOutput")

    # partition = (s, pb); free = (c, fh*W + w) for x, (fh*W + w) for t
    xv = x[:].rearrange("c s (pb fh) w -> (s pb) c (fh w)", pb=PB)  # [128, 4, 8192]
    tv = t[:].rearrange("s o (pb fh) w -> (s o pb) (fh w)", pb=PB)  # [128, 8192]

    AF = mybir.ActivationFunctionType
    OP = mybir.AluOpType
    f32 = mybir.dt.float32
    bf16 = mybir.dt.bfloat16

    with TileContext(nc) as tc:
        with (
            tc.tile_pool(name="const", bufs=1) as constp,
            tc.tile_pool(name="accp", bufs=1) as accp,
            tc.tile_pool(name="xp", bufs=2) as xp,
            tc.tile_pool(name="tp", bufs=2) as tp,
            tc.tile_pool(name="ep", bufs=2) as ep,
            tc.tile_pool(name="up", bufs=2) as up,
            tc.tile_pool(name="lp", bufs=2) as lp,
            tc.tile_pool(name="psum", bufs=2, space="PSUM") as pp,
        ):
            eye = constp.tile([P, P], bf16, name="eye")
            acc = accp.tile([P, NACC], f32, name="acc")

            state = []  # per-chunk tiles to finish one chunk later

            def start_chunk(k):
                sl = slice(k * FC, (k + 1) * FC)
                X4 = xp.tile([P, C, FC], f32, tag="x", name=f"X4_{k}")
                T = tp.tile([P, FC], mybir.dt.int32, tag="t", name=f"T_{k}")
                E = [
                    ep.tile([P, FC], bf16, tag=f"e{c}", name=f"E{c}_{k}")
                    for c in range(C)
                ]
                U = [
                    up.tile([P, FC], bf16, tag=f"u{c}", name=f"U{c}_{k}")
                    for c in range(C)
                ]
                # per-class loads: downstream ops on class c start as soon as
                # its 1MiB lands instead of waiting for the whole chunk
                for c in range(C):
                    nc.sync.dma_start(X4[:, c, :], xv[:, c, sl])
                nc.scalar.dma_start(T[:], tv[:, sl])
                if k == 0:
                    nc.scalar.dma_start(eye[:], eye_d[:])

                # one-hot gather of the LOGIT on DVE: U_c = (t == c) * x_c.
                # Depends only on the DMA, so it runs parallel to ACT's exps.
                for c in range(C):
                    nc.vector.scalar_tensor_tensor(
                        U[c][:], T[:], float(c), X4[:, c, :], OP.is_equal, OP.mult
                    )
                for c in range(C):
                    nc.scalar.activation(E[c][:], X4[:, c, :], AF.Exp)
                # x_t = sum_c U_c (DVE tree, in-place)
                nc.vector.tensor_tensor(U[0][:], U[0][:], U[1][:], OP.add)
                nc.vector.tensor_tensor(U[2][:], U[2][:], U[3][:], OP.add)
                nc.vector.tensor_tensor(U[0][:], U[0][:], U[2][:], OP.add)
                # denominator sum on the tensor engine: D = sum_c E_c
                D = pp.tile([P, FC], f32, tag="d", name=f"D_{k}")
                for j in range(FC // MM):
                    for c in range(C):
                        nc.tensor.matmul(
                            D[:, j * MM : (j + 1) * MM],
                            eye[:],
                            E[c][:, j * MM : (j + 1) * MM],
                            start=(c == 0),
                            stop=(c == C - 1),
                        )
                state.append((k, D, U[0]))

            def finish_chunk():
                k, D, XT = state.pop(0)
                LND = lp.tile([P, FC], bf16, tag="lnd", name=f"LND_{k}")
                ZE = lp.tile([P, FC], bf16, tag="ze", name=f"ZE_{k}")
                nc.scalar.activation(LND[:], D[:], AF.Ln)
                # z = x_t - ln D; p = exp(z), accumulated per partition
                nc.vector.tensor_tensor(XT[:], XT[:], LND[:], OP.subtract)
                nc.scalar.activation(
                    ZE[:], XT[:], AF.Exp, accum_out=acc[:, k : k + 1]
                )

            for k in range(NCHUNK):
                start_chunk(k)
                if k >= 1:
                    finish_chunk()
            finish_chunk()
            nc.scalar.dma_start(out[:, :NCHUNK], acc[:, :NCHUNK])
    nc.compile()  # bacc passes: split sync waits, fill ISA bytes, ...
    _force_single_act_table(nc)
    return nc


def _force_single_act_table(nc):
    """The bacc pass picks the first act-table set per function (Exp->0,
    Ln->5), reloading tables on every switch (~2.7us each). Both live in
    set 6 (natural_log_exp_and_others): retarget and dedupe the loads."""
    both = 6
    for blk in nc.main_func.blocks:
        keep = []
        last = None
        for ins in blk.instructions:
            if type(ins).__name__ == "InstLoadActFuncSet":
                if ins.act_func_set_id in (0, 5):
                    ins.act_func_set_id = both
                if ins.sync_info is None and last == ins.act_func_set_id:
                    continue  # redundant reload
                last = ins.act_func_set_id
            keep.append(ins)
        blk.instructions[:] = keep


def kernel(input, target):
    global LAST_EXEC_NS
    nc = _cache.get("nc")
    if nc is None:
        nc = _cache.setdefault("nc", _build())

    input = np.asarray(input)
    target = np.asarray(target)
    bf16_np = mybir.dt.np(mybir.dt.bfloat16)
    eye_np = np.eye(P, dtype=np.float32).astype(bf16_np)
    in_maps = []
    for i in range(NCORES):
        in_maps.append(
            {
                "x": np.ascontiguousarray(
                    input[i * SPC : (i + 1) * SPC].transpose(1, 0, 2, 3),
                    dtype=np.float32,
                ),
                "t": np.ascontiguousarray(
                    target[i * SPC : (i + 1) * SPC], dtype=np.int32
                ),
                "eye": eye_np,
            }
        )
    res = run_bass_kernel_spmd(nc, in_maps, list(range(NCORES)))
    LAST_EXEC_NS = res.exec_time_ns
    globals()["LAST_RESULT"] = res

    Is = []
    for i in range(NCORES):
        o = np.asarray(res.results[i]["out"], dtype=np.float64)  # [128, NCHUNK]
        Is.append(o.sum(axis=1).reshape(SPC, PB).sum(axis=1))
    intersection = np.concatenate(Is)  # [32]
    hw = float(H * W)
    dice = 2.0 * intersection / (hw + hw + EPS)
    return np.float32(np.mean(1.0 - dice))


# revision 26
# speedup vs baseline: 1.2451x; 1.1157x over previous
"""DiceLoss (softmax + one-hot gather + per-sample dice) on 8 trn2 cores.

Sharding: pure data-parallel over the batch dim (N=32 -> 4 samples/core).
Each core streams its 4 samples, computing per-pixel
    p = exp(x_t) / sum_c exp(x_c)
and accumulating per-partition sums of p. The host finishes with the
(tiny) dice formula. The softmax prob sum over classes is identically 1
per pixel, so cardinality = 2*H*W analytically.

Per-core layout: partitions = (4 samples x 32 pixel-blocks) = 128; free
dim = 8192 pixels per block, processed in 4 chunks of 2048.

Engine assignment (GPSIMD stays idle: its SBUF port lock vs DVE was the
previous bottleneck -- concurrent DVE ops ran 25x slow):
  - DMA:  x as one 4MiB HWDGE transfer per chunk (sync ring);
          t (int32) + out on the scalar HWDGE ring
  - ACT:  exp(x_c) per class (bf16), ln(D)/ln(N) straight from PSUM,
          final exp(lnN-lnD) with accum_out per-partition sums
  - DVE:  fused one-hot numerator U_c=(t==c)*e_c via scalar_tensor_tensor,
          z = lnN - lnD (bf16 2x mode)
  - PE:   class-sum trees D=sum_c E_c, N=sum_c U_c as identity-weight
          matmuls accumulating in PSUM (4 matmuls per 512-col bank)
"""

import os
import sys

import numpy as np


def _ensure_concourse():
    try:
        import concourse.bass  # noqa: F401
    except ImportError:
        for p in (
            "/opt/trn_rl_repo",
            os.path.expanduser("~/.axon_site/_ro/trn_rl_repo"),
        ):
            if os.path.isdir(p) and p not in sys.path:
                sys.path.insert(0, p)


_ensure_concourse()

import concourse.bacc as bacc  # noqa: E402
import concourse.bass as bass  # noqa: E402
import concourse.mybir as mybir  # noqa: E402
from concourse.bass_utils import run_bass_kernel_spmd  # noqa: E402
from concourse.tile import TileContext  # noqa: E402

N, C, H, W = 32, 4, 512, 512
NCORES = 8
SPC = N // NCORES  # samples per core = 4
PB = 32  # pixel blocks per sample (partition sub-dim)
P = SPC * PB  # 128 partitions
FTOT = H * W // PB  # 8192 free-dim pixels per block
FC = 2048  # chunk size along free dim
NCHUNK = FTOT // FC  # 4
# half-size first chunk (faster pipeline fill) and last chunk (shorter
# serial ln->sub->exp tail); middle chunks full size
CHUNKS = [(0, 1024), (1024, 2048), (3072, 2048), (5120, 2048), (7168, 1024)]
NACC = len(CHUNKS)
MM = 512  # matmul output columns per instruction (1 PSUM bank)
EPS = 1e-6

_cache = {}
LAST_EXEC_NS = None
LAST_RESULT = None


def _build():
    nc = bacc.Bacc(None)
    # x arrives class-outermost so the (s, pb) partition dims are adjacent in
    # HBM -> one mergeable partition stride -> one 4MiB DMA per chunk.
    x = nc.dram_tensor("x", [C, SPC, H, W], mybir.dt.float32, kind="ExternalInput")
    t = nc.dram_tensor("t", [SPC, 1, H, W], mybir.dt.int32, kind="ExternalInput")
    eye_d = nc.dram_tensor("eye", [P, P], mybir.dt.bfloat16, kind="ExternalInput")
    out = nc.dram_tensor("out", [P, NACC], mybir.dt.float32, kind="ExternalOutput")

    # partition = (s, pb); free = (c, fh*W + w) for x, (fh*W + w) for t
    xv = x[:].rearrange("c s (pb fh) w -> (s pb) c (fh w)", pb=PB)  # [128, 4, 8192]
    tv = t[:].rearrange("s o (pb fh) w -> (s o pb) (fh w)", pb=PB)  # [128, 8192]

    AF = mybir.ActivationFunctionType
    OP = mybir.AluOpType
    f32 = mybir.dt.float32
    bf16 = mybir.dt.bfloat16

    with TileContext(nc) as tc:
        with (
            tc.tile_pool(name="const", bufs=1) as constp,
            tc.tile_pool(name="accp", bufs=1) as accp,
            tc.tile_pool(name="xp", bufs=2) as xp,
            tc.tile_pool(name="tp", bufs=2) as tp,
            tc.tile_pool(name="ep", bufs=2) as ep,
            tc.tile_pool(name="up", bufs=2) as up,
            tc.tile_pool(name="lp", bufs=2) as lp,
            tc.tile_pool(name="psum", bufs=2, space="PSUM") as pp,
        ):
            eye = constp.tile([P, P], bf16, name="eye")
            acc = accp.tile([P, NACC], f32, name="acc")

            state = []  # per-chunk tiles to finish one chunk later

            def start_chunk(k):
                lo, fc = CHUNKS[k]
                sl = slice(lo, lo + fc)
                X4 = xp.tile([P, C, FC], f32, tag="x", name=f"X4_{k}")
                T = tp.tile([P, FC], mybir.dt.int32, tag="t", name=f"T_{k}")
                E = [
                    ep.tile([P, FC], bf16, tag=f"e{c}", name=f"E{c}_{k}")
                    for c in range(C)
                ]
                U = [
                    up.tile([P, FC], bf16, tag=f"u{c}", name=f"U{c}_{k}")
                    for c in range(C)
                ]
                # per-class loads: downstream ops on class c start as soon as
                # its 1MiB lands instead of waiting for the whole chunk
                for c in range(C):
                    nc.sync.dma_start(X4[:, c, :fc], xv[:, c, sl])
                nc.scalar.dma_start(T[:, :fc], tv[:, sl])
                if k == 0:
                    nc.scalar.dma_start(eye[:], eye_d[:])

                # one-hot gather of the LOGIT on DVE: U_c = (t == c) * x_c.
                # Depends only on the DMA, so it runs parallel to ACT's exps.
                for c in range(C):
                    nc.vector.scalar_tensor_tensor(
                        U[c][:, :fc], T[:, :fc], float(c), X4[:, c, :fc],
                        OP.is_equal, OP.mult,
                    )
                for c in range(C):
                    nc.scalar.activation(E[c][:, :fc], X4[:, c, :fc], AF.Exp)
                # x_t = sum_c U_c (DVE tree, in-place)
                nc.vector.tensor_tensor(U[0][:, :fc], U[0][:, :fc], U[1][:, :fc], OP.add)
                nc.vector.tensor_tensor(U[2][:, :fc], U[2][:, :fc], U[3][:, :fc], OP.add)
                nc.vector.tensor_tensor(U[0][:, :fc], U[0][:, :fc], U[2][:, :fc], OP.add)
                # denominator sum on the tensor engine: D = sum_c E_c
                D = pp.tile([P, FC], f32, tag="d", name=f"D_{k}")
                for j in range(fc // MM):
                    for c in range(C):
                        nc.tensor.matmul(
                            D[:, j * MM : (j + 1) * MM],
                            eye[:],
                            E[c][:, j * MM : (j + 1) * MM],
                            start=(c == 0),
                            stop=(c == C - 1),
                        )
                state.append((k, fc, D, U[0]))

            def finish_chunk():
                k, fc, D, XT = state.pop(0)
                LND = lp.tile([P, FC], bf16, tag="lnd", name=f"LND_{k}")
                ZE = lp.tile([P, FC], bf16, tag="ze", name=f"ZE_{k}")
                nc.scalar.activation(LND[:, :fc], D[:, :fc], AF.Ln)
                # z = x_t - ln D; p = exp(z), accumulated per partition
                nc.vector.tensor_tensor(XT[:, :fc], XT[:, :fc], LND[:, :fc], OP.subtract)
                nc.scalar.activation(
                    ZE[:, :fc], XT[:, :fc], AF.Exp, accum_out=acc[:, k : k + 1]
                )
                if k == NACC - 2:
                    # all but the last column: off the postamble critical path
                    nc.scalar.dma_start(out[:, : NACC - 1], acc[:, : NACC - 1])
                elif k == NACC - 1:
                    nc.scalar.dma_start(out[:, k : k + 1], acc[:, k : k + 1])

            for k in range(NACC):
                start_chunk(k)
                if k >= 1:
                    finish_chunk()
            finish_chunk()
    nc.compile()  # bacc passes: split sync waits, fill ISA bytes, ...
    _force_single_act_table(nc)
    return nc


def _force_single_act_table(nc):
    """The bacc pass picks the first act-table set per function (Exp->0,
    Ln->5), reloading tables on every switch (~2.7us each). Both live in
    set 6 (natural_log_exp_and_others): retarget and dedupe the loads."""
    both = 6
    for blk in nc.main_func.blocks:
        keep = []
        last = None
        for ins in blk.instructions:
            if type(ins).__name__ == "InstLoadActFuncSet":
                if ins.act_func_set_id in (0, 5):
                    ins.act_func_set_id = both
                if ins.sync_info is None and last == ins.act_func_set_id:
                    continue  # redundant reload
                last = ins.act_func_set_id
            keep.append(ins)
        blk.instructions[:] = keep


def kernel(input, target):
    global LAST_EXEC_NS
    nc = _cache.get("nc")
    if nc is None:
        nc = _cache.setdefault("nc", _build())

    input = np.asarray(input)
    target = np.asarray(target)
    bf16_np = mybir.dt.np(mybir.dt.bfloat16)
    eye_np = np.eye(P, dtype=np.float32).astype(bf16_np)
    in_maps = []
    for i in range(NCORES):
        in_maps.append(
            {
                "x": np.ascontiguousarray(
                    input[i * SPC : (i + 1) * SPC].transpose(1, 0, 2, 3),
                    dtype=np.float32,
                ),
                "t": np.ascontiguousarray(
                    target[i * SPC : (i + 1) * SPC], dtype=np.int32
                ),
                "eye": eye_np,
            }
        )
    res = run_bass_kernel_spmd(nc, in_maps, list(range(NCORES)))
    LAST_EXEC_NS = res.exec_time_ns
    globals()["LAST_RESULT"] = res

    Is = []
    for i in range(NCORES):
        o = np.asarray(res.results[i]["out"], dtype=np.float64)  # [128, NCHUNK]
        Is.append(o.sum(axis=1).reshape(SPC, PB).sum(axis=1))
    intersection = np.concatenate(Is)  # [32]
    hw = float(H * W)
    dice = 2.0 * intersection / (hw + hw + EPS)
    return np.float32(np.mean(1.0 - dice))
